# revision 29
# baseline (speedup 1.0000x reference)
"""Trainium2 Bass kernel for nn_Encoder_7413113553686.

Key algebraic fact exploited: the reference loops
    out = x0
    for i in range(L): out = _guidance(x0, q_w[i], kv_w[i], proj_w[i], proj_b[i])
where every iteration consumes the SAME x0 — so the result is just the LAST
block (i = L-1 = 20) applied to x0.  Everything else is dead compute.

Sharding over 8 cores: core c = (b, g) with b = c>>1 (batch), g = c&1
(head-group: heads 4g..4g+3).  Each core computes x0 for its batch, its 4
heads of attention, and a partial projection.  Both cores add 0.5*x0 so the
host pair-sum reconstructs the residual; proj bias is added on the host.

Trace-driven design (all numbers measured on TRN2):
  * bf16 everywhere: 512-free matmuls issue at 215ns (1 col/cycle); f32r
    ran as fp32_mode=HIGH at 4/3 cyc/col with 225ns LDWEIGHTS.
  * LayerNorm: mean-centering folded into conv weights on the host;
    variance via the 65x65 quadratic form var = p^T (W''W''^T/D) p; eps
    folded into G's ones-row corner.  Conv runs ONCE (ACT evicts).
  * Both ACT table sets (Ln set + Exp set) warmed at boot — the Exp set
    load otherwise lands mid-chain for 1.3us.
  * Next-head q/k GEMMs spread 3 matmuls per score-tile iteration
    (instead of one 24-matmul burst at m==3): the softmax exp is 1147ns
    per tile vs 860ns of scores+attnV PE work, so the burst left the
    inner loop exp-bound; spreading makes every m PE-bound (~1.5us).
  * x0 assembly split DVE (m0-3) / gpsimd (m4-5): 12 tensor_tensor ops
    at ~0.7us would otherwise serialize 8.3us on DVE.
  * proj residual fused into eviction: out = (x0*0.5) + psum via one
    scalar_tensor_tensor; proj inner order puts h0-2 matmuls before the
    norm3-gated h3 ones so they prestart during the norm3 chain.
  * PSUM (8 banks): phase A = dm(1)+quad(2, tag-ring)+conv(4); attention
    = big 2x[128,1024](4) + acc 1x(2, pv/po share) + qks 2x[96,512](2).
  * ~3us of boot warm-up matmuls + pinned dummies hold the PE HAM
    activity window (cold PE = 1.2 GHz).
  * DMA queues: sync = pT,G,wcg,qw,kw,vw; vector = pos0-2;
    scalar = pos3-5; gpsimd = pw.  (Single-queue pos on the SWDGE path
    measured ~100GB/s and gated x0 by ~6us.)
"""

import os
import sys

import numpy as np

for _p in ("/opt/trn_rl_repo",):
    if os.path.isdir(_p) and _p not in sys.path:
        sys.path.insert(0, _p)

from concourse import bacc, bass, mybir, tile  # noqa: E402
from concourse.bass_utils import run_bass_kernel_spmd  # noqa: E402

F32 = mybir.dt.float32
MM = mybir.dt.bfloat16
NPBF16 = mybir.dt.np(MM)

B, D, N, NH, HD = 4, 768, 1024, 8, 96
SCALE = float(HD) ** -0.5
LAYER = 20
AF = mybir.ActivationFunctionType
ALU = mybir.AluOpType


def _body(nc, tc, io, outT):
    mm = nc.tensor.matmul

    import contextlib
    _persist_ctx = contextlib.ExitStack()
    persist = _persist_ctx.enter_context(
        tc.tile_pool(name="persist", bufs=1))

    def ptile(name, shape, dtype=MM):
        return persist.tile(shape, dtype, tag=name, name=name)

    # ---------------- boot: constants + ACT table warm-up ----------------
    junk = ptile("junk", [128, 64], MM)
    nc.gpsimd.memset(junk[:, :], 0.125)
    ones65 = ptile("ones65", [65, 1], MM)
    nc.gpsimd.memset(ones65[:, :], 1.0)
    onesr = ptile("onesr", [1, 128], MM)
    nc.gpsimd.memset(onesr[:, :], 1.0)
    eps_col = ptile("eps_col", [1, 1], F32)
    nc.gpsimd.memset(eps_col[:, :], 1e-5)
    v_sb = [ptile(f"v{m}", [128, 4 * 97], MM) for m in range(8)]
    for m in range(8):
        v3 = v_sb[m].rearrange("p (h d) -> p h d", h=4)
        nc.gpsimd.memset(v3[:, :, 96:97], 1.0)
    # ---------------- input DMAs, two queues, ordered by first use -------
    # The SDMA engines heavily favor whichever queue enqueues first, so
    # the sync queue carries ONLY the phase-A chain (wcg_g -> pT -> pos)
    # and the scalar queue's weight transfers are GATED behind pT's
    # completion by a tiny dependent ACT copy.  G rides inside wcg_g
    # (standalone 130-byte rows measured sub-line-rate).
    sb_wg = ptile("sb_wg", [65, D + 65], MM)   # [wcg | G]
    nc.sync.dma_start(out=sb_wg[:, :], in_=io["wcg_g"][:, :])
    sb_wcg = sb_wg[:, 0:D]
    sb_G = sb_wg[:, D:D + 65]
    sb_pT = ptile("sb_pT", [65, N], MM)
    nc.sync.dma_start(out=sb_pT[:, :], in_=io["pT"][:, :])
    pos_big = ptile("pos_big", [128, 6 * N], MM)
    for third in range(3):
        nc.sync.dma_start(
            out=pos_big.rearrange("p (m c) -> p m c", m=6)[:, 2 * third:2 * third + 2, :],
            in_=io["posT"].rearrange("(m p) c -> p m c", m=6)[:, 2 * third:2 * third + 2, :])
    pos_sb = [pos_big[:, m * N:(m + 1) * N] for m in range(6)]

    # Ln and Exp live in different ACT table sets; warm both FIRST (the
    # loads finish before the gate releases, so nothing lands mid-chain).
    warm_ln = ptile("warm_ln", [1, 1], F32)
    nc.scalar.activation(warm_ln[:, :], eps_col[:, :], AF.Ln)
    warm_exp = ptile("warm_exp", [1, 1], F32)
    nc.scalar.activation(warm_exp[:, :], eps_col[:, :], AF.Exp)
    # gate: holds the scalar queue's weight DMAs until pT has landed
    gate = ptile("gate", [1, 8], MM)
    nc.scalar.copy(gate[:, :], sb_pT[0:1, 0:8])

    qkv_big = {}
    pw_big = ptile("pw_big", [96, 4 * D], MM)
    for nm in ("qw", "kw", "vw"):
        qkv_big[nm] = ptile(f"{nm}_big", [128, 6 * 384], MM)

    def emit_weight_dma(nm):
        if nm == "pw":
            nc.scalar.dma_start(
                out=pw_big.rearrange("p (h c) -> p h c", h=4),
                in_=io["pw"].rearrange("(h p) c -> p h c", h=4))
        else:
            nc.scalar.dma_start(
                out=qkv_big[nm].rearrange("p (k c) -> p k c", k=6),
                in_=io[nm].rearrange("(k p) c -> p k c", k=6))

    emit_weight_dma("qw")  # vw/pw issued later, after the Ln/Exp chain
    emit_weight_dma("kw")  # kw early: the fused q0/k0/q1 trickle needs it
    qw_sb = [qkv_big["qw"][:, k * 384:(k + 1) * 384] for k in range(6)]
    kw_sb = [qkv_big["kw"][:, k * 384:(k + 1) * 384] for k in range(6)]
    vw_sb = [qkv_big["vw"][:, k * 384:(k + 1) * 384] for k in range(6)]
    pw_sb = [pw_big[:, h * D:(h + 1) * D] for h in range(4)]

    # persistent activations
    x0T = [ptile(f"x0T{m}", [128, N], MM) for m in range(6)]
    xc = [ptile(f"xc{m}", [128, N], MM) for m in range(6)]
    oT = [ptile(f"oT{h}", [96, N], MM) for h in range(4)]
    srow = ptile("srow", [128, N], F32)  # head h uses partition h*32
    rstd_b = ptile("rstd_b", [128, N], MM)
    rstd_row = ptile("rstd_row", [1, N], MM)

    # ---------------- phase A: conv + LN stats + x0 ----------------
    with (
        tc.tile_pool(name="dm", bufs=1, space="PSUM") as dmp,
        tc.tile_pool(name="qd", bufs=1, space="PSUM") as qd,
        tc.tile_pool(name="cv", bufs=2, space="PSUM") as cv,
        tc.tile_pool(name="wkA", bufs=2) as wkA,
    ):
        dpsum = dmp.tile([64, 512], F32, name="dpsum")

        def dummy(rhs):
            # tiny matmul reading `rhs` — pins PE activity to that tensor's
            # readiness so the HAM busy-window never sees a >3.4us idle gap
            mm(dpsum[0:64, 0:rhs.shape[-1]], junk[0:rhs.shape[0], 0:64],
               rhs, start=True, stop=True)

        for _ in range(56):
            mm(dpsum[:, 0:64], junk[:, 0:64], junk[:, 0:64],
               start=True, stop=True)

        # var(token) = p^T G p  (G = W''W''^T/D + eps on the ones corner)
        ps_tmp = qd.tile([65, N], F32, tag="q", name="ps_tmp")
        for n in range(2):
            sl = bass.ts(n, 512)
            mm(ps_tmp[:, sl], sb_G[:, :], sb_pT[:, sl], start=True, stop=True)

        pcs = {}

        def conv(m):
            pc = cv.tile([128, N], F32, tag="pc", name=f"pc{m}")
            for n in range(2):
                sl = bass.ts(n, 512)
                mm(pc[:, sl], sb_wcg[:, m * 128:(m + 1) * 128], sb_pT[:, sl],
                   start=True, stop=True)
            nc.scalar.copy(xc[m][:, :], pc[:, :])

        conv(0)
        conv(1)

        pm = wkA.tile([65, N], MM, tag="pm", name="pm", bufs=1)
        nc.vector.tensor_mul(pm[:, :], ps_tmp[:, :], sb_pT[:, :])
        ps_ss = qd.tile([1, N], F32, tag="q", name="ps_ss")
        for n in range(2):
            sl = bass.ts(n, 512)
            mm(ps_ss[:, sl], ones65[:, :], pm[:, sl], start=True, stop=True)
        # rstd = exp(-0.5 * ln(var + eps))  (eps already inside ps_ss)
        lnrow = wkA.tile([1, N], F32, tag="lnrow", name="lnrow", bufs=1)
        nc.scalar.activation(lnrow[:, :], ps_ss[:, :], AF.Ln)
        with nc.allow_low_precision(reason="rstd in bf16"):
            nc.scalar.activation(rstd_row[:, :], lnrow[:, :], AF.Exp,
                                 scale=-0.5)
        # broadcast rstd over partitions (gpsimd; its queue is light now)
        nc.gpsimd.partition_broadcast(rstd_b[:, :], rstd_row[:, :])
        # remaining weight DMAs issue here: after the Ln/Exp chain in the
        # ACT FIFO (ahead of it they would delay rstd by ~3us), but early
        # enough that kw/vw/pw land before their first use
        for nm in ("vw", "pw"):
            emit_weight_dma(nm)

        for m in range(2, 6):
            conv(m)
        dummy(rstd_row[:, 0:512])

        # x0 = xc * rstd + (ln_b + pos): DVE only — gpsimd tensor_tensor
        # measured 2108ns/tile (3x slower than DVE's 683ns)
        for m in range(6):
            t = wkA.tile([128, N], MM, tag="t", name=f"t{m}", bufs=2)
            nc.vector.tensor_mul(t[:, :], xc[m][:, :], rstd_b[:, :])
            nc.vector.tensor_add(x0T[m][:, :], t[:, :], pos_sb[m][:, :])
            if m == 0:
                dummy(x0T[0][:, 0:512])

    # ---------------- attention ----------------
    with (
        tc.tile_pool(name="ps", bufs=2, space="PSUM") as ps,
        tc.tile_pool(name="wk", bufs=2) as wk,
        tc.tile_pool(name="expp", bufs=3) as expp,
    ):
        qT_t = [wk.tile([96, N], MM, tag="qT", name=f"qT{h}", bufs=4)
                for h in range(4)]
        kT_t = [wk.tile([96, N], MM, tag="kT", name=f"kT{h}", bufs=4)
                for h in range(4)]

        def emit_qk01():
            # q0/k0/q1 accumulate TOGETHER, k-chunk by k-chunk, as the x0
            # chunks land: 6 matmuls per chunk keeps the PE ~saturated
            # through the x0 DVE drum (so HAM warms early).  3 PSUM
            # accumulators = big(2) + acc(1); k1 runs after, dense.
            pq0 = ps.tile([96, N], F32, tag="big", name="pq0")
            pk0 = ps.tile([96, N], F32, tag="big", name="pk0")
            pq1 = ps.tile([96, N], F32, tag="acc", name="pq1", bufs=1)
            for k in range(6):
                for t, w, hs in ((pq0, qw_sb, slice(0, 96)),
                                 (pq1, qw_sb, slice(96, 192)),
                                 (pk0, kw_sb, slice(0, 96))):
                    for n in range(2):
                        sl = bass.ts(n, 512)
                        mm(t[:, sl], w[k][:, hs], x0T[k][:, sl],
                           start=(k == 0), stop=(k == 5))
            # evictions on ACT: the DVE is the x0 drum right now
            nc.scalar.copy(qT_t[0][:, :], pq0[:, :])
            nc.scalar.copy(kT_t[0][:, :], pk0[:, :])
            nc.scalar.copy(qT_t[1][:, :], pq1[:, :])
            pk1 = ps.tile([96, N], F32, tag="big", name="pk1")
            for n in range(2):
                sl = bass.ts(n, 512)
                for k in range(6):
                    mm(pk1[:, sl], kw_sb[k][:, 96:192], x0T[k][:, sl],
                       start=(k == 0), stop=(k == 5))
            nc.scalar.copy(kT_t[1][:, :], pk1[:, :])

        # q2/k2/q3/k3 chunks spread 2 matmuls per score-tile iteration
        # across heads 0-2 (24 m-iterations x 2 = 48 = 4 sub-tiles x 2
        # halves x 6 k): keeps every head PE-bound instead of exp-bound.
        qk_jobs = []   # flat list of (dst_tile, wsel, hs, k, start, stop)
        for h_next in (2, 3):
            hs = slice(h_next * 96, (h_next + 1) * 96)
            for quarter in range(4):
                wsel = qw_sb if quarter < 2 else kw_sb
                nsel = quarter % 2
                dst = qT_t[h_next] if quarter < 2 else kT_t[h_next]
                for k in range(6):
                    qk_jobs.append((h_next, quarter, nsel, wsel, hs, dst, k))
        qk_state = {}

        def qk_chunk(job_i):
            h_next, quarter, nsel, wsel, hs, dst, k = qk_jobs[job_i]
            sl = bass.ts(nsel, 512)
            if k == 0:
                qk_state['sub'] = ps.tile([96, 512], F32, tag="qks",
                                          name=f"qks{h_next}_{quarter}")
            sub = qk_state['sub']
            mm(sub[:, :], wsel[k][:, hs], x0T[k][:, sl],
               start=(k == 0), stop=(k == 5))
            if k == 5:
                nc.vector.tensor_copy(dst[:, sl], sub[:, :])

        def emit_norm(h):
            # softmax denominators: spread the [1,1024] row over 128
            # partitions via a reshape DMA, reciprocal at full width,
            # DMA back, broadcast, scale oT in place
            s_pk = wk.tile([128, 8], F32, tag="spk", name=f"spk{h}", bufs=1)
            nc.sync.dma_start(out=s_pk[:, :], in_=srow[h * 32:h * 32 + 1, :])
            r_pk = wk.tile([128, 8], MM, tag="rpk", name=f"rpk{h}", bufs=1)
            with nc.allow_low_precision(reason="softmax denom recip bf16"):
                nc.vector.reciprocal(r_pk[:, :], s_pk[:, :])
            recip = wk.tile([1, N], MM, tag="row2", name=f"rc{h}", bufs=1)
            nc.sync.dma_start(out=recip[:, :], in_=r_pk[:, :])
            if h < 3:
                rb = wk.tile([96, N], MM, tag="rb", name=f"rb{h}", bufs=1)
                nc.gpsimd.partition_broadcast(rb[:, :], recip[:, :])
                nc.vector.tensor_mul(oT[h][:, :], oT[h][:, :], rb[:, :])
            else:
                # last head gates the projection: broadcast via K=1 matmuls
                # into the (now free) qks slots — faster than gpsimd here
                for n in range(2):
                    sl = bass.ts(n, 512)
                    rb3 = ps.tile([96, 512], F32, tag="qks", name=f"rb3_{n}")
                    mm(rb3[:, :], onesr[:, 0:96], recip[:, sl],
                       start=True, stop=True)
                    nc.vector.tensor_mul(oT[3][:, sl], oT[3][:, sl],
                                         rb3[:, :])

        emit_qk01()
        # V tiles (dense, before the head loop: pv and po share one acc slot)
        for m in range(8):
            pv = ps.tile([128, 384], F32, tag="acc", name=f"pv{m}", bufs=1)
            for k in range(6):
                mm(pv[:, :], x0T[k][:, m * 128:(m + 1) * 128], vw_sb[k][:, :],
                   start=(k == 0), stop=(k == 5))
            v3 = v_sb[m].rearrange("p (h d) -> p h d", h=4)
            nc.vector.tensor_copy(
                v3[:, :, 0:96], pv.rearrange("p (h d) -> p h d", h=4))

        pp0 = None
        job_i = 0
        for h in range(4):
            qT, kT = qT_t[h], kT_t[h]
            po = ps.tile([97, N], F32, tag="acc", name=f"po{h}", bufs=1)
            for m in range(8):
                pss = ps.tile([128, N], F32, tag="big", name=f"pss{h}_{m}")
                for n in range(2):
                    sl = bass.ts(n, 512)
                    mm(pss[:, sl], kT[:, m * 128:(m + 1) * 128], qT[:, sl],
                       start=True, stop=True)
                ex = expp.tile([128, N], MM, tag="exp", name=f"ex{h}_{m}")
                nc.scalar.activation(ex[:, :], pss[:, :], AF.Exp)
                if m == 0 and h >= 1:
                    emit_norm(h - 1)  # previous head's normalize, overlapped
                # q2/k2/q3/k3 chunks sit in the exp->attnV latency window
                if h < 3:
                    qk_chunk(job_i)
                    qk_chunk(job_i + 1)
                    job_i += 2
                if m == 6 and h == 3:
                    # pre-start proj m0: everything except the h3 term
                    pp0 = ps.tile([128, N], F32, tag="big", name="pp0")
                    for n2 in range(2):
                        sl2 = bass.ts(n2, 512)
                        for hh in range(3):
                            mm(pp0[:, sl2], pw_sb[hh][:, 0:128],
                               oT[hh][:, sl2], start=(hh == 0), stop=False)
                v3 = v_sb[m].rearrange("p (h d) -> p h d", h=4)
                for n in range(2):
                    sl = bass.ts(n, 512)
                    mm(po[:, sl], v3[:, h, :], ex[:, sl],
                       start=(m == 0), stop=(m == 7))
            # evict po fast: denominator row first (feeds the reciprocal),
            # numerators via ACT (it has slack between exps)
            nc.vector.tensor_copy(srow[h * 32:h * 32 + 1, :], po[96:97, :])
            nc.scalar.copy(oT[h][:, :], po[0:96, :])
        # pp1's h0-2 matmuls BEFORE emit_norm(3): the rb3 matmuls inside
        # it wait on the reciprocal chain and would head-of-line block the
        # (data-ready) pp1 work in the in-order PE FIFO

        # ---------------- proj; residual fused into the eviction --------
        # Ladder: pp[m+1]'s h0-2 matmuls are emitted BEFORE pp[m]'s h3
        # finish.  The PE FIFO is in-order, and pp[m]-h3 waits the norm3
        # chain — without the ladder everything queues behind it, the PE
        # idles ~4.5us, HAM re-throttles, and the whole proj runs at
        # 1.2 GHz.  With it the h0-2 work fills the norm3 window.
        pps = [pp0] + [None] * 5

        def proj_h012(m, tag="big"):
            # pp2/pp4 ride the acc slot (freed once po3 evicts): the big
            # ring only fits one prestart tile alongside the held pp0
            pps[m] = ps.tile([128, N], F32, tag=tag, name=f"pp{m}",
                             bufs=1 if tag == "acc" else 2)
            for n in range(2):
                sl = bass.ts(n, 512)
                for hh in range(3):
                    mm(pps[m][:, sl], pw_sb[hh][:, m * 128:(m + 1) * 128],
                       oT[hh][:, sl], start=(hh == 0), stop=False)

        def proj_fin(m):
            for n in range(2):
                sl = bass.ts(n, 512)
                mm(pps[m][:, sl], pw_sb[3][:, m * 128:(m + 1) * 128],
                   oT[3][:, sl], start=False, stop=True)
            ou = wk.tile([128, N], MM, tag="out", name=f"ou{m}")
            # out = 0.5*x0 + proj_psum (pair-sum on host restores 1.0*x0)
            nc.vector.scalar_tensor_tensor(
                ou[:, :], x0T[m][:, :], 0.5, pps[m][:, :],
                op0=ALU.mult, op1=ALU.add)
            eng = nc.sync if m % 2 == 1 else nc.scalar
            eng.dma_start(out=outT[m * 128:(m + 1) * 128, :], in_=ou[:, :])

        proj_h012(1)
        proj_h012(2, tag="acc")  # before emit_norm(3): the rb3 matmuls
        emit_norm(3)             # wait the recip chain and would block it
        proj_fin(0)
        proj_h012(3)
        proj_fin(1)
        proj_h012(4, tag="acc")
        proj_fin(2)
        proj_h012(5)
        for m in range(3, 6):
            proj_fin(m)


def _build_nc():
    nc = bacc.Bacc("TRN2", target_bir_lowering=False, debug=False,
                   enable_asserts=False)
    io = {}
    for name, shape, dt in (
        ("pT", [65, N], MM), ("wcg_g", [65, D + 65], MM),
        ("posT", [D, N], MM), ("qw", [D, 384], MM), ("kw", [D, 384], MM),
        ("vw", [D, 384], MM), ("pw", [384, D], MM),
    ):
        io[name] = nc.dram_tensor(name, shape, dt, kind="ExternalInput").ap()
    outT = nc.dram_tensor("outT", [D, N], MM, kind="ExternalOutput").ap()
    with tile.TileContext(nc) as tc:
        _body(nc, tc, io, outT)
    nc.compile()
    return nc


_NC_CACHE = {}


def _get_nc():
    if "nc" not in _NC_CACHE:
        _NC_CACHE["nc"] = _build_nc()
    return _NC_CACHE["nc"]


def _prep_in_maps(sam, conv_w, conv_b, ln_g, ln_b, pos, q_w, kv_w, proj_w,
                  proj_b):
    f = np.float32
    sam = np.asarray(sam, f)
    qwL = (np.asarray(q_w[LAYER], f) * SCALE).astype(f)
    kvL = np.asarray(kv_w[LAYER], f)
    kwL, vwL = kvL[:, :D], kvL[:, D:]
    pwL = np.ascontiguousarray(np.asarray(proj_w[LAYER], f))

    g = np.asarray(ln_g, f)
    # centering folded into the conv weights: x - mean_d(x) = W'' p with
    # W''[c,d] = W[c,d] - mean_d(W[c,:]); gamma folded on top.  Variance
    # uses the un-scaled W'' via G = W''W''^T/D; LN eps rides on the
    # ones-row corner of G (p_aug[64] == 1 always).
    W2 = np.asarray(conv_w, f).reshape(D, 64).T            # [64, 768]
    Wc = np.concatenate([W2, np.asarray(conv_b, f)[None, :]], 0)  # [65, 768]
    Wpp = Wc - Wc.mean(axis=1, keepdims=True)
    G = (Wpp @ Wpp.T) / D                                  # [65, 65]
    G[64, 64] += 1e-5
    wcg_g = np.ascontiguousarray(
        np.concatenate([Wpp * g[None, :], G], axis=1))     # [65, 833]

    posT_eff = np.ascontiguousarray(
        np.asarray(ln_b, f)[:, None] + np.asarray(pos, f).T)  # [768, 1024]

    bf = NPBF16
    in_maps = []
    for c in range(8):
        b, gi = c >> 1, c & 1
        img = sam[b, 0]
        patches = img.reshape(32, 8, 32, 8).transpose(0, 2, 1, 3).reshape(1024, 64)
        pT_aug = np.ascontiguousarray(
            np.concatenate([patches.T, np.ones((1, N), f)], 0))  # [65, 1024]
        sl = slice(gi * 384, (gi + 1) * 384)
        in_maps.append({
            "pT": pT_aug.astype(bf),
            "wcg_g": wcg_g.astype(bf),
            "posT": posT_eff.astype(bf),
            "qw": np.ascontiguousarray(qwL[:, sl]).astype(bf),
            "kw": np.ascontiguousarray(kwL[:, sl]).astype(bf),
            "vw": np.ascontiguousarray(vwL[:, sl]).astype(bf),
            "pw": np.ascontiguousarray(pwL[sl, :]).astype(bf),
        })
    return in_maps


def kernel(sam, conv_w, conv_b, ln_g, ln_b, pos, q_w, kv_w, proj_w, proj_b,
           **_unused):
    nc = _get_nc()
    in_maps = _prep_in_maps(sam, conv_w, conv_b, ln_g, ln_b, pos, q_w, kv_w,
                            proj_w, proj_b)
    res = run_bass_kernel_spmd(nc, in_maps, core_ids=list(range(8)))
    outs = [np.asarray(r["outT"], dtype=np.float32) for r in res.results]
    pbL = np.asarray(proj_b[LAYER], np.float32)
    full = np.stack(
        [(outs[2 * b] + outs[2 * b + 1]).T + pbL[None, :] for b in range(B)])
    return np.ascontiguousarray(full.astype(np.float32))


if __name__ == "__main__":
    # quick smoke test against the reference when run in the problem dir
    sys.path.insert(0, os.path.dirname(os.path.abspath(__file__)))
    import reference as R

    inputs = {k: np.asarray(v) for k, v in R.setup_inputs().items()}
    expected = np.asarray(R.reference(**inputs))
    actual = kernel(**inputs)
    rel = np.linalg.norm(actual - expected) / np.linalg.norm(expected)
    print("Relative error:", rel)
